# revision 1
# baseline (speedup 1.0000x reference)
"""HNet energy kernel v4: contraction-outer, 2 output passes, paced DMA.

Math (exact): energies[i,j] = null_count[j] + sum_v (temp==tmap[v]).(L==v),
output = energies - min(energies).  null/min/add on host (exact fp32).

Per core (4 point-groups x 2 cmp-groups): the contraction over 64 edge
chunks is the OUTER loop; 8 PSUM banks hold 8 output tiles (2 cmp tiles
x 4 point tiles) for a whole pass, and 2 passes cover the 4 cmp tiles.
A-mask blocks (host-staged, exact {0,1} fp8) and L blocks stream through
small tile pools, so every DMA is gated by matmul consumption (~75 GB/s
steady) -- an unpaced multi-MB DMA burst trips the board throttle and
locks the PE to ~0.83x clock for the rest of the run (measured v2/v3).
B masks are built on device: DVE is_equal, or an exact one-relu ACT
indicator for the alphabet max 9.
"""

import numpy as np
import ml_dtypes

import concourse.bacc as bacc
import concourse.mybir as mybir
from concourse.tile import TileContext
from concourse.bass_utils import run_bass_kernel_spmd

# ---- problem constants (hardcoded from spec) ----
N_PTS, N_NODES, N_EDGES, N_CMP = 2048, 1024, 8192, 4096
PGROUPS, CGROUPS = 4, 2          # 8 cores = 4 point-groups x 2 cmp-groups
P = N_PTS // PGROUPS             # 512 points per core
C = N_CMP // CGROUPS             # 2048 cmp columns per core
ECHUNKS = N_EDGES // 128         # 64 edge chunks of 128
NTILES = C // 512                # 4 cmp tiles of 512 per core
MTILES = P // 128                # 4 point chunks of 128 per core
ABLK = 4                         # edge chunks per streamed block
NBLKS = ECHUNKS // ABLK          # 16 blocks
NPASS = 2                        # output passes (2 cmp tiles each)

FP8 = mybir.dt.float8e4
F32 = mybir.dt.float32
NP_FP8 = ml_dtypes.float8_e4m3
DR = mybir.MatmulPerfMode.DoubleRow
EQ = mybir.AluOpType.is_equal
RELU = mybir.ActivationFunctionType.Relu
LMAX = 9                         # EDG alphabet max

_CODE2TEMP = {2: 0, 3: 1, 5: 2, 9: 3}   # EDG code value -> temp index

_nc_cache: dict = {}


def _build_nc(pairs):
    """Build the SPMD Bass program. pairs = tuple of (temp_val, L_val)."""
    npair = len(pairs)
    nc = bacc.Bacc(None)
    #   Am : [npair, 128, ECHUNKS*P]   [q, ki, ec*P+p] =
    #           (temp[pg*P+p, ec*128+ki] == tmap[v_q])
    #   LT : [NTILES, NBLKS, 128, ABLK*512] [nt, blk, ki, c*512+j] =
    #           L[cg*C+nt*512+j, (blk*ABLK+c)*128+ki]
    Am = nc.dram_tensor("Am", [npair, 128, ECHUNKS * P], FP8,
                        kind="ExternalInput")
    LT = nc.dram_tensor("LT", [NTILES, NBLKS, 128, ABLK * 512], FP8,
                        kind="ExternalInput")
    en = nc.dram_tensor("en", [P, C], F32, kind="ExternalOutput")

    with TileContext(nc) as tc:
        with (
            tc.tile_pool(name="const", bufs=1) as const_pool,
            tc.tile_pool(name="a", bufs=3 * npair) as a_pool,
            tc.tile_pool(name="b", bufs=6 * npair) as b_pool,
            tc.tile_pool(name="lt", bufs=6) as lt_pool,
            tc.tile_pool(name="out", bufs=8) as out_pool,
            tc.tile_pool(name="psum", bufs=8, space="PSUM") as psum_pool,
        ):
            bias_m8 = const_pool.tile([128, 1], F32, tag="bias")
            nc.any.memset(bias_m8[:], float(1 - LMAX))
            # pre-warm the ACT function table during the framework preamble
            # (first ACTIVATE otherwise pays a ~1.3us table load on the
            # first-matmul critical path)
            warm = const_pool.tile([128, 1], F32, tag="warm")
            nc.scalar.activation(warm[:], bias_m8[:], RELU,
                                 bias=bias_m8[:], scale=1.0)
            # PE clock warmup: dummy matmuls in the otherwise-idle preamble
            # window so the ramp to full clock happens before real data lands
            dummy = const_pool.tile([128, 2, 512], FP8, tag="dummy")
            nc.any.memset(dummy[:], 0.0)
            wps = psum_pool.tile([128, 512], F32, name="wps", tag="ps")
            for w in range(7):
                nc.tensor.matmul(wps, lhsT=dummy[:, :, 0:128], rhs=dummy[:],
                                 start=(w == 0), stop=(w == 6), perf_mode=DR)

            def drain(p_prev, eps_prev, final=False):
                """Copy finished PSUM tiles + DMA out.  Mid-run: DVE only,
                so ACT keeps masking for the next pass.  Final: alternate
                ACT/DVE (ACT is idle) to halve the copy chain."""
                for ntl in range(2):
                    nt = 2 * p_prev + ntl
                    for m in range(MTILES):
                        ot = out_pool.tile([128, 512], F32, name="ot",
                                           tag="out")
                        if final and m % 2 == 0:
                            nc.scalar.copy(out=ot[:], in_=eps_prev[ntl][m][:])
                        else:
                            nc.vector.tensor_copy(ot[:], eps_prev[ntl][m][:])
                        # split the ~0.6us/issue chains across both
                        # HWDGE queues (copies stay off ACT mid-run, so its
                        # mask stream is not interrupted)
                        eng = nc.scalar if m % 2 == 1 else nc.sync
                        eng.dma_start(
                            out=en[m * 128:(m + 1) * 128,
                                   nt * 512:(nt + 1) * 512],
                            in_=ot[:])

            prev = None
            for p in range(NPASS):
                eps = [[psum_pool.tile([128, 512], F32, name="ep", tag="ps")
                        for _m in range(MTILES)] for _ntl in range(2)]
                for blk in range(NBLKS):
                    a_t = [a_pool.tile([128, ABLK, P], FP8, name="at",
                                       tag="a") for _q in range(npair)]
                    lts = [lt_pool.tile([128, ABLK, 512], FP8, name="lt",
                                        tag="lt") for _ntl in range(2)]
                    def mask_into(dst, src, lv):
                        if lv == LMAX:
                            nc.scalar.activation(dst, src, RELU,
                                                 bias=bias_m8[:], scale=1.0)
                        else:
                            nc.vector.tensor_scalar(
                                out=dst, in0=src, scalar1=float(lv),
                                scalar2=None, op0=EQ)

                    bt = [[None] * 2 for _q in range(npair)]
                    if p == 0 and blk == 0:
                        # startup critical path: the first chunk-pair of
                        # q0's A + ntl0's L land (and mask) first, in small
                        # slices; LT via the ACT HWDGE queue, in parallel
                        # with the serialized ~0.6us/issue Sync queue
                        nc.scalar.dma_start(out=lts[0][:, 0:2, :],
                                            in_=LT[0, 0, :, 0:1024])
                        nc.sync.dma_start(out=a_t[0][:, 0:2, :],
                                          in_=Am[0, :, 0:2 * P])
                        nc.sync.dma_start(out=lts[0][:, 2:4, :],
                                          in_=LT[0, 0, :, 1024:2048])
                        nc.sync.dma_start(out=a_t[0][:, 2:4, :],
                                          in_=Am[0, :, 2 * P:4 * P])
                        for q, (_tv, lv) in enumerate(pairs):
                            d = b_pool.tile([128, ABLK, 512], FP8, name="bt",
                                            tag="b")
                            mask_into(d[:, 0:2, :], lts[0][:, 0:2, :], lv)
                            bt[q][0] = d
                        for q in range(1, npair):
                            nc.sync.dma_start(
                                out=a_t[q][:],
                                in_=Am[q, :,
                                       blk * ABLK * P:(blk + 1) * ABLK * P])
                        nc.sync.dma_start(out=lts[1][:],
                                          in_=LT[2 * p + 1, blk])
                        for q, (_tv, lv) in enumerate(pairs):
                            mask_into(bt[q][0][:, 2:4, :],
                                      lts[0][:, 2:4, :], lv)
                        for q, (_tv, lv) in enumerate(pairs):
                            d = b_pool.tile([128, ABLK, 512], FP8, name="bt",
                                            tag="b")
                            mask_into(d[:], lts[1][:], lv)
                            bt[q][1] = d
                    else:
                        for q in range(npair):
                            nc.sync.dma_start(
                                out=a_t[q][:],
                                in_=Am[q, :,
                                       blk * ABLK * P:(blk + 1) * ABLK * P])
                        for ntl in range(2):
                            nc.sync.dma_start(out=lts[ntl][:],
                                              in_=LT[2 * p + ntl, blk])
                        for ntl in range(2):
                            for q, (_tv, lv) in enumerate(pairs):
                                d = b_pool.tile([128, ABLK, 512], FP8,
                                                name="bt", tag="b")
                                mask_into(d[:], lts[ntl][:], lv)
                                bt[q][ntl] = d
                    if prev is not None and blk == 0:
                        drain(*prev)
                    for cp in range(ABLK // 2):
                        first = (blk == 0 and cp == 0)
                        last = (blk == NBLKS - 1 and cp == ABLK // 2 - 1)
                        if last:
                            # bank-major: each bank's stop matmul lands as
                            # early as possible so copies/DMAs pipeline
                            order = [(q, ntl, m) for ntl in range(2)
                                     for m in range(MTILES)
                                     for q in range(npair)]
                        else:
                            order = [(q, ntl, m) for q in range(npair)
                                     for ntl in range(2)
                                     for m in range(MTILES)]
                        for q, ntl, m in order:
                            nc.tensor.matmul(
                                eps[ntl][m],
                                lhsT=a_t[q][:, 2 * cp:2 * cp + 2,
                                            m * 128:(m + 1) * 128],
                                rhs=bt[q][ntl][:, 2 * cp:2 * cp + 2, :],
                                start=(first and q == 0),
                                stop=(last and q == npair - 1),
                                perf_mode=DR)
                prev = (p, eps)
            drain(*prev, final=True)
    if not nc.is_finalized():
        nc.finalize()
    return nc


def _get_nc(pairs):
    key = tuple(pairs)
    if key not in _nc_cache:
        _nc_cache[key] = _build_nc(key)
    return _nc_cache[key]


def _prep_inputs(node_activations, learned_edge_states, edge_endnode_idx,
                 pairs):
    na = np.asarray(node_activations)
    L = np.asarray(learned_edge_states, dtype=np.float32)
    idx = np.asarray(edge_endnode_idx)
    npair = len(pairs)

    # temp[i,e] in {0,1,2,3}; transposed view [e, i]
    tempT = (2 * na[:, idx[:, 0]] + na[:, idx[:, 1]]).astype(np.int8).T

    # A masks: big[q][ki, ec, pt] = (temp[pt, ec*128+ki] == tv)
    a_big = []
    for tv, _lv in pairs:
        m = (tempT == tv)                       # [N_EDGES, N_PTS] bool
        a_big.append(m.reshape(ECHUNKS, 128, N_PTS).transpose(1, 0, 2))

    # L transposed/tiled: lt_big[cg*NTILES+nt, blk, ki, c, j]
    lt_big = (L.T.reshape(NBLKS, ABLK, 128, CGROUPS * NTILES, 512)
              .transpose(3, 0, 2, 1, 4))

    null_count = (L == 0.0).sum(axis=1).astype(np.float32)   # [N_CMP]

    lt_cores = []
    for cg in range(CGROUPS):
        LTc = np.empty((NTILES, NBLKS, 128, ABLK * 512), dtype=NP_FP8)
        for nt in range(NTILES):
            for blk in range(NBLKS):
                LTc[nt, blk] = np.ascontiguousarray(
                    lt_big[cg * NTILES + nt, blk]).astype(
                        NP_FP8).reshape(128, ABLK * 512)
        lt_cores.append(LTc)

    in_maps = []
    for pg in range(PGROUPS):
        Am = np.empty((npair, 128, ECHUNKS * P), dtype=NP_FP8)
        for q in range(npair):
            Am[q] = np.ascontiguousarray(
                a_big[q][:, :, pg * P:(pg + 1) * P]).astype(
                    NP_FP8).reshape(128, ECHUNKS * P)
        for cg in range(CGROUPS):
            in_maps.append({"Am": Am, "LT": lt_cores[cg]})
    return in_maps, null_count


def _kept_pairs(edge_type_filter):
    seen = []
    for v in np.asarray(edge_type_filter).ravel().tolist():
        v = int(v)
        if v in _CODE2TEMP and v not in [p[1] for p in seen]:
            seen.append((_CODE2TEMP[v], v))
    # DVE-masked pairs first: the first matmul then gates on the faster
    # DVE is_equal while the ACT relu indicator runs in parallel
    seen.sort(key=lambda p: p[1] == LMAX)
    return tuple(seen)


def kernel(node_activations, learned_edge_states, edge_endnode_idx,
           edge_type_filter, _trace=False, _tmpdir=None):
    pairs = _kept_pairs(edge_type_filter)
    L = np.asarray(learned_edge_states, dtype=np.float32)
    if len(pairs) == 0:
        # nothing kept: energies are null_count rows broadcast
        null_count = (L == 0.0).sum(axis=1).astype(np.float32)
        en = np.broadcast_to(null_count[None, :], (N_PTS, N_CMP)).copy()
        return en - en.min()

    nc = _get_nc(pairs)
    in_maps, null_count = _prep_inputs(node_activations, learned_edge_states,
                                       edge_endnode_idx, pairs)
    res = run_bass_kernel_spmd(nc, in_maps, core_ids=list(range(8)),
                               trace=_trace, tmpdir=_tmpdir)
    out = np.empty((N_PTS, N_CMP), dtype=np.float32)
    for ci in range(8):
        pg, cg = ci // CGROUPS, ci % CGROUPS
        out[pg * P:(pg + 1) * P, cg * C:(cg + 1) * C] = res.results[ci]["en"]
    out += null_count[None, :]
    out -= out.min()
    if _trace:
        kernel._last_results = res
    return out



# revision 4
# speedup vs baseline: 1.6416x; 1.6416x over previous
"""HNet energy kernel v5: algebraic reduction to ONE fp8-DR GEMM stream.

Math (exact): es one-hot indicators decompose over the 2-bit temp
alphabet: A_t(a0,a1) = c_t + l0_t*a0 + l1_t*a1 + q_t*a0*a1 with
q_t in {+1,-1}.  Summing over kept edge codes v:

  energies[i,j] = const[j] + sum_n na[i,n] * W[j,n]
                 + sum_e AND[i,e] * Q[j,e]

with AND[i,e] = na[i,n0[e]] & na[i,n1[e]], Q in {-1,0,1} (disjoint
one-hots), W integer (scatter-add of per-endpoint counts; split into
fp8-exact parts with |w|<=16 if needed), const[j] = null_count +
sum c_t cnt_v.  That's 8192+1024 contraction instead of 2x8192:
576 DR matmuls/core instead of 1024, with ZERO on-device masking
(all operands host-built exact fp8).

Per core (4 point-groups x 2 cmp-groups): lhs XT ([128, 72*512] fp8,
36KB/partition) is RESIDENT in SBUF, DMA'd blockwise during pass 0 and
reused in pass 1.  rhs R blocks stream through a pool, DMA gated by
matmul consumption (pacing: unpaced bursts trip the board DMA throttle
-> PE locked at ~0.83x clock, measured v2/v3).  Pass 1 runs its two
cmp-tiles as sequential phases over resident rhs tiles so the first
half of the final output drains (copy+DMA wire time) under the last
~31us of compute; only 4 banks drain after the last matmul.
"""

import numpy as np
import ml_dtypes

import concourse.bacc as bacc
import concourse.mybir as mybir
from concourse.tile import TileContext
from concourse.bass_utils import run_bass_kernel_spmd

# ---- problem constants (hardcoded from spec) ----
N_PTS, N_NODES, N_EDGES, N_CMP = 2048, 1024, 8192, 4096
PGROUPS, CGROUPS = 4, 2          # 8 cores = 4 point-groups x 2 cmp-groups
P = N_PTS // PGROUPS             # 512 points per core
C = N_CMP // CGROUPS             # 2048 cmp columns per core
ECHUNKS = N_EDGES // 128         # 64 edge chunks of 128
NCHUNKS = N_NODES // 128         # 8 node chunks of 128 per W part
NTILES = C // 512                # 4 cmp tiles of 512 per core
MTILES = P // 128                # 4 point chunks of 128 per core
ABLK = 4                         # contraction chunks per streamed block
NPASS = 2                        # output passes (2 cmp tiles each)

FP8 = mybir.dt.float8e4
F32 = mybir.dt.float32
NP_FP8 = ml_dtypes.float8_e4m3
DR = mybir.MatmulPerfMode.DoubleRow

_CODE2TEMP = {2: 0, 3: 1, 5: 2, 9: 3}   # EDG code value -> temp index
# A_t(a0,a1) = c + l0*a0 + l1*a1 + q*a0*a1
_COEF = {0: (1, -1, -1, 1), 1: (0, 0, 1, -1),
         2: (0, 1, 0, -1), 3: (0, 0, 0, 1)}

_nc_cache: dict = {}


def _build_nc(nparts):
    """SPMD Bass program.  Contraction = 64 edge chunks + 8*nparts node
    chunks, all fp8 DoubleRow, 8 PSUM banks per pass, 2 passes."""
    nchunk = ECHUNKS + NCHUNKS * nparts
    nblks = nchunk // ABLK
    nc = bacc.Bacc(None)
    #   XT : [128, nchunk*P]          [ki, c*P+p] = X[pg*P+p, c*128+ki]
    #   RT : [NTILES, nblks, 128, ABLK*512]
    #        [nt, blk, ki, c*512+j] = R[cg*C+nt*512+j, (blk*ABLK+c)*128+ki]
    XT = nc.dram_tensor("XT", [128, nchunk * P], FP8, kind="ExternalInput")
    RT = nc.dram_tensor("RT", [NTILES, nblks, 128, ABLK * 512], FP8,
                        kind="ExternalInput")
    en = nc.dram_tensor("en", [P, C], F32, kind="ExternalOutput")

    with TileContext(nc) as tc:
        with (
            tc.tile_pool(name="const", bufs=1) as const_pool,
            tc.tile_pool(name="x", bufs=1) as x_pool,
            tc.tile_pool(name="rt", bufs=18) as rt_pool,
            tc.tile_pool(name="out", bufs=8) as out_pool,
            tc.tile_pool(name="psum", bufs=8, space="PSUM") as psum_pool,
        ):
            # ACT table prewarm (first ACTIVATE otherwise pays ~1.3us
            # table load when the mid-run drain copies start)
            warm = const_pool.tile([128, 1], F32, tag="warm")
            nc.any.memset(warm[:], 0.0)
            nc.scalar.copy(out=warm[:], in_=warm[:])
            # PE clock warmup: dummy matmuls in the otherwise-idle
            # preamble window so the HAM ramp to full clock happens
            # before real data lands
            dummy = const_pool.tile([128, 2, 512], FP8, tag="dummy")
            nc.any.memset(dummy[:], 0.0)
            wps = psum_pool.tile([128, 512], F32, name="wps", tag="ps")
            for w in range(7):
                nc.tensor.matmul(wps, lhsT=dummy[:, :, 0:128], rhs=dummy[:],
                                 start=(w == 0), stop=(w == 6), perf_mode=DR)

            xt = x_pool.tile([128, nchunk, P], FP8, tag="x")

            def drain(banks, final=False):
                """banks: list of (nt, m, ps) in completion order.  Only
                DVE (vector) and ACT (scalar) can read PSUM; out DMAs go
                on the sync + gpsimd queues so the copy engines' chains
                are not interrupted by ~0.6us DMA issues."""
                cengs = (nc.vector, nc.scalar)
                dengs = (nc.sync, nc.gpsimd)
                for k, (nt, m, ps) in enumerate(banks):
                    ot = out_pool.tile([128, 512], F32, name="ot", tag="out")
                    ceng = cengs[k % len(cengs)]
                    if ceng is nc.scalar:
                        nc.scalar.copy(out=ot[:], in_=ps[:])
                    else:
                        ceng.tensor_copy(ot[:], ps[:])
                    dengs[k % 2].dma_start(
                        out=en[m * 128:(m + 1) * 128,
                               nt * 512:(nt + 1) * 512],
                        in_=ot[:])

            # ---- pass 0: cmp tiles 0,1 interleaved; xt streams in ----
            eps0 = [[psum_pool.tile([128, 512], F32, name="ep", tag="ps")
                     for _m in range(MTILES)] for _ntl in range(2)]
            for blk in range(nblks):
                base = blk * ABLK
                rts = [rt_pool.tile([128, ABLK, 512], FP8, name="rt",
                                    tag="rt") for _ntl in range(2)]
                if blk == 0:
                    # startup critical path: first chunk-pair of XT and
                    # RT[0,0] land first, in halves, on separate queues
                    nc.scalar.dma_start(out=xt[:, 0:2, :],
                                        in_=XT[:, 0:2 * P])
                    nc.sync.dma_start(out=rts[0][:, 0:2, :],
                                      in_=RT[0, 0, :, 0:1024])
                    nc.scalar.dma_start(out=xt[:, 2:4, :],
                                        in_=XT[:, 2 * P:4 * P])
                    nc.sync.dma_start(out=rts[0][:, 2:4, :],
                                      in_=RT[0, 0, :, 1024:2048])
                    nc.sync.dma_start(out=rts[1][:], in_=RT[1, 0])
                else:
                    nc.scalar.dma_start(
                        out=xt[:, base:base + ABLK, :],
                        in_=XT[:, base * P:(base + ABLK) * P])
                    for ntl in range(2):
                        nc.sync.dma_start(out=rts[ntl][:], in_=RT[ntl, blk])
                for cp in range(ABLK // 2):
                    first = (blk == 0 and cp == 0)
                    last = (blk == nblks - 1 and cp == ABLK // 2 - 1)
                    if first:
                        # rts[1] arrives later: run all ntl0 work first
                        order = [(ntl, m) for ntl in range(2)
                                 for m in range(MTILES)]
                    elif last:
                        # bank-major so each bank's stop lands early and
                        # drain copies pipeline with the last matmuls
                        order = [(ntl, m) for ntl in range(2)
                                 for m in range(MTILES)]
                    else:
                        # m-outer: consecutive ntl pair shares lhsT
                        order = [(ntl, m) for m in range(MTILES)
                                 for ntl in range(2)]
                    for ntl, m in order:
                        nc.tensor.matmul(
                            eps0[ntl][m],
                            lhsT=xt[:, base + 2 * cp:base + 2 * cp + 2,
                                    m * 128:(m + 1) * 128],
                            rhs=rts[ntl][:, 2 * cp:2 * cp + 2, :],
                            start=first, stop=last, perf_mode=DR)

            # ---- pass 1: cmp tiles 2,3 as sequential phases over ----
            # ---- resident rhs tiles; xt already resident           ----
            eps1 = [[psum_pool.tile([128, 512], F32, name="ep", tag="ps")
                     for _m in range(MTILES)] for _ntl in range(2)]
            drain([(ntl, m, eps0[ntl][m])
                   for ntl in range(2) for m in range(MTILES)])
            for ntl in range(2):
                for blk in range(nblks):
                    base = blk * ABLK
                    rt = rt_pool.tile([128, ABLK, 512], FP8, name="rt",
                                      tag="rt")
                    nc.sync.dma_start(out=rt[:], in_=RT[2 + ntl, blk])
                    for cp in range(ABLK // 2):
                        first = (blk == 0 and cp == 0)
                        last = (blk == nblks - 1 and cp == ABLK // 2 - 1)
                        for m in range(MTILES):
                            nc.tensor.matmul(
                                eps1[ntl][m],
                                lhsT=xt[:, base + 2 * cp:base + 2 * cp + 2,
                                        m * 128:(m + 1) * 128],
                                rhs=rt[:, 2 * cp:2 * cp + 2, :],
                                start=first, stop=last, perf_mode=DR)
                if ntl == 0:
                    # phase 0 banks drain under phase 1's ~31us compute
                    drain([(2, m, eps1[0][m]) for m in range(MTILES)])
            drain([(3, m, eps1[1][m]) for m in range(MTILES)], final=True)
    if not nc.is_finalized():
        nc.finalize()
    return nc


def _get_nc(nparts):
    if nparts not in _nc_cache:
        _nc_cache[nparts] = _build_nc(nparts)
    return _nc_cache[nparts]


def _segsum(B, col_idx, n_cols):
    """out[j, n] = sum_{e: col_idx[e]==n} B[j, e]  (B float32 [J, E])."""
    E = B.shape[1]
    perm = np.argsort(col_idx, kind="stable")
    starts = np.searchsorted(col_idx[perm], np.arange(n_cols))
    out = np.add.reduceat(B[:, perm], np.minimum(starts, E - 1), axis=1)
    counts = np.bincount(col_idx, minlength=n_cols)
    out[:, counts == 0] = 0.0
    return out


def _host_terms(na, L, idx, kept):
    """Build const [N_CMP], W [N_CMP, N_NODES], Q [N_CMP, N_EDGES]."""
    const = (L == 0.0).sum(axis=1).astype(np.float64)
    W = np.zeros((N_CMP, N_NODES), np.float32)
    Q = np.zeros((N_CMP, N_EDGES), np.float32)
    for v in kept:
        t = _CODE2TEMP[v]
        c, l0, l1, q = _COEF[t]
        Bv = (L == float(v)).astype(np.float32)
        if c:
            const = const + c * Bv.sum(axis=1, dtype=np.float64)
        Q += q * Bv
        for k, lk in ((0, l0), (1, l1)):
            if lk:
                W += lk * _segsum(Bv, idx[:, k], N_NODES)
    return const.astype(np.float32), W, Q


def _split_w(W):
    """Exact fp8-e4m3 split: W = sum(parts), each part integer |w|<=16."""
    parts = []
    R = W.copy()
    while np.any(R):
        part = np.clip(R, -16.0, 16.0)
        parts.append(part)
        R = R - part
    return parts


def _prep_inputs(na, L, idx, kept):
    nau8 = na.astype(np.uint8)
    const, W, Q = _host_terms(nau8, L, idx, kept)
    wparts = _split_w(W)
    nparts = len(wparts)
    nchunk = ECHUNKS + NCHUNKS * nparts
    nblks = nchunk // ABLK

    AND = (nau8[:, idx[:, 0]] & nau8[:, idx[:, 1]]).astype(np.float32)
    X = np.concatenate([AND] + [nau8.astype(np.float32)] * nparts, axis=1)
    R = np.concatenate([Q] + wparts, axis=1)

    xt_pgs = []
    for pg in range(PGROUPS):
        xs = X[pg * P:(pg + 1) * P]
        xt_pgs.append(np.ascontiguousarray(
            xs.reshape(P, nchunk, 128).transpose(2, 1, 0)).astype(
                NP_FP8).reshape(128, nchunk * P))
    rt_cgs = []
    for cg in range(CGROUPS):
        rs = R[cg * C:(cg + 1) * C]
        rt_cgs.append(np.ascontiguousarray(
            rs.reshape(NTILES, 512, nblks, ABLK, 128)
            .transpose(0, 2, 4, 3, 1)).astype(
                NP_FP8).reshape(NTILES, nblks, 128, ABLK * 512))

    in_maps = []
    for pg in range(PGROUPS):
        for cg in range(CGROUPS):
            in_maps.append({"XT": xt_pgs[pg], "RT": rt_cgs[cg]})
    return in_maps, const, nparts


def _kept_vals(edge_type_filter):
    seen = []
    for v in np.asarray(edge_type_filter).ravel().tolist():
        v = int(v)
        if v in _CODE2TEMP and v not in seen:
            seen.append(v)
    return seen


def kernel(node_activations, learned_edge_states, edge_endnode_idx,
           edge_type_filter, _trace=False, _tmpdir=None):
    na = np.asarray(node_activations)
    L = np.asarray(learned_edge_states, dtype=np.float32)
    idx = np.asarray(edge_endnode_idx)
    kept = _kept_vals(edge_type_filter)
    if len(kept) == 0:
        null_count = (L == 0.0).sum(axis=1).astype(np.float32)
        en = np.broadcast_to(null_count[None, :], (N_PTS, N_CMP)).copy()
        return en - en.min()

    in_maps, const, nparts = _prep_inputs(na, L, idx, kept)
    nc = _get_nc(nparts)
    res = run_bass_kernel_spmd(nc, in_maps, core_ids=list(range(8)),
                               trace=_trace, tmpdir=_tmpdir)
    out = np.empty((N_PTS, N_CMP), dtype=np.float32)
    for ci in range(8):
        pg, cg = ci // CGROUPS, ci % CGROUPS
        out[pg * P:(pg + 1) * P, cg * C:(cg + 1) * C] = res.results[ci]["en"]
    out += const[None, :]
    out -= out.min()
    if _trace:
        kernel._last_results = res
    return out


# revision 7
# speedup vs baseline: 1.6580x; 1.0100x over previous
"""HNet energy kernel v5: algebraic reduction to ONE fp8-DR GEMM stream.

Math (exact): es one-hot indicators decompose over the 2-bit temp
alphabet: A_t(a0,a1) = c_t + l0_t*a0 + l1_t*a1 + q_t*a0*a1 with
q_t in {+1,-1}.  Summing over kept edge codes v:

  energies[i,j] = const[j] + sum_n na[i,n] * W[j,n]
                 + sum_e AND[i,e] * Q[j,e]

with AND[i,e] = na[i,n0[e]] & na[i,n1[e]], Q in {-1,0,1} (disjoint
one-hots), W integer (scatter-add of per-endpoint counts; split into
fp8-exact parts with |w|<=16 if needed), const[j] = null_count +
sum c_t cnt_v.  That's 8192+1024 contraction instead of 2x8192:
576 DR matmuls/core instead of 1024, with ZERO on-device masking
(all operands host-built exact fp8).

Per core (4 point-groups x 2 cmp-groups): lhs XT ([128, 72*512] fp8,
36KB/partition) is RESIDENT in SBUF, DMA'd blockwise during pass 0 and
reused in pass 1.  rhs R blocks stream through a pool, DMA gated by
matmul consumption (pacing: unpaced bursts trip the board DMA throttle
-> PE locked at ~0.83x clock, measured v2/v3).  Pass 1 runs its two
cmp-tiles as sequential phases over resident rhs tiles so the first
half of the final output drains (copy+DMA wire time) under the last
~31us of compute; only 4 banks drain after the last matmul.
"""

import numpy as np
import ml_dtypes

import concourse.bacc as bacc
import concourse.mybir as mybir
from concourse.tile import TileContext
from concourse.bass_utils import run_bass_kernel_spmd

# ---- problem constants (hardcoded from spec) ----
N_PTS, N_NODES, N_EDGES, N_CMP = 2048, 1024, 8192, 4096
PGROUPS, CGROUPS = 4, 2          # 8 cores = 4 point-groups x 2 cmp-groups
P = N_PTS // PGROUPS             # 512 points per core
C = N_CMP // CGROUPS             # 2048 cmp columns per core
ECHUNKS = N_EDGES // 128         # 64 edge chunks of 128
NCHUNKS = N_NODES // 128         # 8 node chunks of 128 per W part
NTILES = C // 512                # 4 cmp tiles of 512 per core
MTILES = P // 128                # 4 point chunks of 128 per core
ABLK = 4                         # contraction chunks per streamed block
NPASS = 2                        # output passes (2 cmp tiles each)

FP8 = mybir.dt.float8e4
F32 = mybir.dt.float32
NP_FP8 = ml_dtypes.float8_e4m3
DR = mybir.MatmulPerfMode.DoubleRow

_CODE2TEMP = {2: 0, 3: 1, 5: 2, 9: 3}   # EDG code value -> temp index
# A_t(a0,a1) = c + l0*a0 + l1*a1 + q*a0*a1
_COEF = {0: (1, -1, -1, 1), 1: (0, 0, 1, -1),
         2: (0, 1, 0, -1), 3: (0, 0, 0, 1)}

_nc_cache: dict = {}


def _build_nc(nparts):
    """SPMD Bass program.  Contraction = 64 edge chunks + 8*nparts node
    chunks, all fp8 DoubleRow, 8 PSUM banks per pass, 2 passes."""
    nchunk = ECHUNKS + NCHUNKS * nparts
    nblks = nchunk // ABLK
    nc = bacc.Bacc(None)
    #   XT : [128, nchunk*P]          [ki, c*P+p] = X[pg*P+p, c*128+ki]
    #   RT : [NTILES, nblks, 128, ABLK*512]
    #        [nt, blk, ki, c*512+j] = R[cg*C+nt*512+j, (blk*ABLK+c)*128+ki]
    XT = nc.dram_tensor("XT", [128, nchunk * P], FP8, kind="ExternalInput")
    RT = nc.dram_tensor("RT", [NTILES, nblks, 128, ABLK * 512], FP8,
                        kind="ExternalInput")
    en = nc.dram_tensor("en", [P, C], F32, kind="ExternalOutput")

    with TileContext(nc) as tc:
        with (
            tc.tile_pool(name="const", bufs=1) as const_pool,
            tc.tile_pool(name="x", bufs=1) as x_pool,
            tc.tile_pool(name="rt", bufs=18) as rt_pool,
            tc.tile_pool(name="out", bufs=8) as out_pool,
            tc.tile_pool(name="psum", bufs=8, space="PSUM") as psum_pool,
        ):
            # PE clock warmup FIRST: dummy matmuls in the otherwise-idle
            # preamble window so the HAM ramp to full clock happens
            # before real data lands
            dummy = const_pool.tile([128, 2, 512], FP8, tag="dummy")
            nc.any.memset(dummy[:], 0.0)
            wps = psum_pool.tile([128, 512], F32, name="wps", tag="ps")
            for w in range(7):
                nc.tensor.matmul(wps, lhsT=dummy[:, :, 0:128], rhs=dummy[:],
                                 start=(w == 0), stop=(w == 6), perf_mode=DR)
            # ACT table prewarm (first ACTIVATE otherwise pays ~1.3us
            # table load when the mid-run drain copies start)
            warm = const_pool.tile([128, 1], F32, tag="warm")
            nc.any.memset(warm[:], 0.0)
            nc.scalar.copy(out=warm[:], in_=warm[:])

            xt = x_pool.tile([128, nchunk, P], FP8, tag="x")

            def drain(banks, final=False):
                """banks: list of (nt, m, ps) in completion order.  Only
                DVE (vector) and ACT (scalar) can read PSUM; mid-run the
                out DMAs go on the sync + gpsimd queues so the copy
                engines' chains are not interrupted by ~0.6us DMA
                issues.  The FINAL drain's wire time is the exposed
                tail: spread its 4 DMAs over 4 idle queues."""
                cengs = (nc.vector, nc.scalar)
                dengs = ((nc.sync, nc.gpsimd, nc.scalar, nc.sync)
                         if final else (nc.sync, nc.gpsimd))
                for k, (nt, m, ps) in enumerate(banks):
                    ot = out_pool.tile([128, 512], F32, name="ot", tag="out")
                    ceng = cengs[k % len(cengs)]
                    if ceng is nc.scalar:
                        nc.scalar.copy(out=ot[:], in_=ps[:])
                    else:
                        ceng.tensor_copy(ot[:], ps[:])
                    dengs[k % len(dengs)].dma_start(
                        out=en[m * 128:(m + 1) * 128,
                               nt * 512:(nt + 1) * 512],
                        in_=ot[:])

            # ---- pass 0: cmp tiles 0,1 interleaved; xt streams in ----
            eps0 = [[psum_pool.tile([128, 512], F32, name="ep", tag="ps")
                     for _m in range(MTILES)] for _ntl in range(2)]
            for blk in range(nblks):
                base = blk * ABLK
                rts = [rt_pool.tile([128, ABLK, 512], FP8, name="rt",
                                    tag="rt") for _ntl in range(2)]
                if blk == 0:
                    # startup critical path: first chunk-pair of XT and
                    # RT[0,0] land first, in halves, on separate queues
                    nc.scalar.dma_start(out=xt[:, 0:2, :],
                                        in_=XT[:, 0:2 * P])
                    nc.sync.dma_start(out=rts[0][:, 0:2, :],
                                      in_=RT[0, 0, :, 0:1024])
                    nc.scalar.dma_start(out=xt[:, 2:4, :],
                                        in_=XT[:, 2 * P:4 * P])
                    nc.sync.dma_start(out=rts[0][:, 2:4, :],
                                      in_=RT[0, 0, :, 1024:2048])
                    nc.sync.dma_start(out=rts[1][:], in_=RT[1, 0])
                else:
                    nc.scalar.dma_start(
                        out=xt[:, base:base + ABLK, :],
                        in_=XT[:, base * P:(base + ABLK) * P])
                    for ntl in range(2):
                        nc.sync.dma_start(out=rts[ntl][:], in_=RT[ntl, blk])
                for cp in range(ABLK // 2):
                    first = (blk == 0 and cp == 0)
                    last = (blk == nblks - 1 and cp == ABLK // 2 - 1)
                    if first:
                        # rts[1] arrives later: run all ntl0 work first
                        order = [(ntl, m) for ntl in range(2)
                                 for m in range(MTILES)]
                    elif last:
                        # bank-major so each bank's stop lands early and
                        # drain copies pipeline with the last matmuls
                        order = [(ntl, m) for ntl in range(2)
                                 for m in range(MTILES)]
                    else:
                        # m-outer: consecutive ntl pair shares lhsT
                        order = [(ntl, m) for m in range(MTILES)
                                 for ntl in range(2)]
                    for ntl, m in order:
                        nc.tensor.matmul(
                            eps0[ntl][m],
                            lhsT=xt[:, base + 2 * cp:base + 2 * cp + 2,
                                    m * 128:(m + 1) * 128],
                            rhs=rts[ntl][:, 2 * cp:2 * cp + 2, :],
                            start=first, stop=last, perf_mode=DR)

            # ---- pass 1: cmp tiles 2,3 as sequential phases over ----
            # ---- resident rhs tiles; xt already resident           ----
            eps1 = [[psum_pool.tile([128, 512], F32, name="ep", tag="ps")
                     for _m in range(MTILES)] for _ntl in range(2)]
            drain([(ntl, m, eps0[ntl][m])
                   for ntl in range(2) for m in range(MTILES)])
            for ntl in range(2):
                for blk in range(nblks):
                    base = blk * ABLK
                    rt = rt_pool.tile([128, ABLK, 512], FP8, name="rt",
                                      tag="rt")
                    nc.sync.dma_start(out=rt[:], in_=RT[2 + ntl, blk])
                    for cp in range(ABLK // 2):
                        first = (blk == 0 and cp == 0)
                        last = (blk == nblks - 1 and cp == ABLK // 2 - 1)
                        for m in range(MTILES):
                            nc.tensor.matmul(
                                eps1[ntl][m],
                                lhsT=xt[:, base + 2 * cp:base + 2 * cp + 2,
                                        m * 128:(m + 1) * 128],
                                rhs=rt[:, 2 * cp:2 * cp + 2, :],
                                start=first, stop=last, perf_mode=DR)
                if ntl == 0:
                    # phase 0 banks drain under phase 1's ~31us compute
                    drain([(2, m, eps1[0][m]) for m in range(MTILES)])
            drain([(3, m, eps1[1][m]) for m in range(MTILES)], final=True)
    if not nc.is_finalized():
        nc.finalize()
    return nc


def _get_nc(nparts):
    if nparts not in _nc_cache:
        _nc_cache[nparts] = _build_nc(nparts)
    return _nc_cache[nparts]


def _segsum(B, col_idx, n_cols):
    """out[j, n] = sum_{e: col_idx[e]==n} B[j, e]  (B float32 [J, E])."""
    E = B.shape[1]
    perm = np.argsort(col_idx, kind="stable")
    starts = np.searchsorted(col_idx[perm], np.arange(n_cols))
    out = np.add.reduceat(B[:, perm], np.minimum(starts, E - 1), axis=1)
    counts = np.bincount(col_idx, minlength=n_cols)
    out[:, counts == 0] = 0.0
    return out


def _host_terms(na, L, idx, kept):
    """Build const [N_CMP], W [N_CMP, N_NODES], Q [N_CMP, N_EDGES]."""
    const = (L == 0.0).sum(axis=1).astype(np.float64)
    W = np.zeros((N_CMP, N_NODES), np.float32)
    Q = np.zeros((N_CMP, N_EDGES), np.float32)
    for v in kept:
        t = _CODE2TEMP[v]
        c, l0, l1, q = _COEF[t]
        Bv = (L == float(v)).astype(np.float32)
        if c:
            const = const + c * Bv.sum(axis=1, dtype=np.float64)
        Q += q * Bv
        for k, lk in ((0, l0), (1, l1)):
            if lk:
                W += lk * _segsum(Bv, idx[:, k], N_NODES)
    return const.astype(np.float32), W, Q


def _split_w(W):
    """Exact fp8-e4m3 split: W = sum(parts), each part integer |w|<=16."""
    parts = []
    R = W.copy()
    while np.any(R):
        part = np.clip(R, -16.0, 16.0)
        parts.append(part)
        R = R - part
    return parts


def _prep_inputs(na, L, idx, kept):
    nau8 = na.astype(np.uint8)
    const, W, Q = _host_terms(nau8, L, idx, kept)
    wparts = _split_w(W)
    nparts = len(wparts)
    nchunk = ECHUNKS + NCHUNKS * nparts
    nblks = nchunk // ABLK

    AND = (nau8[:, idx[:, 0]] & nau8[:, idx[:, 1]]).astype(np.float32)
    X = np.concatenate([AND] + [nau8.astype(np.float32)] * nparts, axis=1)
    R = np.concatenate([Q] + wparts, axis=1)

    xt_pgs = []
    for pg in range(PGROUPS):
        xs = X[pg * P:(pg + 1) * P]
        xt_pgs.append(np.ascontiguousarray(
            xs.reshape(P, nchunk, 128).transpose(2, 1, 0)).astype(
                NP_FP8).reshape(128, nchunk * P))
    rt_cgs = []
    for cg in range(CGROUPS):
        rs = R[cg * C:(cg + 1) * C]
        rt_cgs.append(np.ascontiguousarray(
            rs.reshape(NTILES, 512, nblks, ABLK, 128)
            .transpose(0, 2, 4, 3, 1)).astype(
                NP_FP8).reshape(NTILES, nblks, 128, ABLK * 512))

    in_maps = []
    for pg in range(PGROUPS):
        for cg in range(CGROUPS):
            in_maps.append({"XT": xt_pgs[pg], "RT": rt_cgs[cg]})
    return in_maps, const, nparts


def _kept_vals(edge_type_filter):
    seen = []
    for v in np.asarray(edge_type_filter).ravel().tolist():
        v = int(v)
        if v in _CODE2TEMP and v not in seen:
            seen.append(v)
    return seen


def kernel(node_activations, learned_edge_states, edge_endnode_idx,
           edge_type_filter, _trace=False, _tmpdir=None):
    na = np.asarray(node_activations)
    L = np.asarray(learned_edge_states, dtype=np.float32)
    idx = np.asarray(edge_endnode_idx)
    kept = _kept_vals(edge_type_filter)
    if len(kept) == 0:
        null_count = (L == 0.0).sum(axis=1).astype(np.float32)
        en = np.broadcast_to(null_count[None, :], (N_PTS, N_CMP)).copy()
        return en - en.min()

    in_maps, const, nparts = _prep_inputs(na, L, idx, kept)
    nc = _get_nc(nparts)
    res = run_bass_kernel_spmd(nc, in_maps, core_ids=list(range(8)),
                               trace=_trace, tmpdir=_tmpdir)
    out = np.empty((N_PTS, N_CMP), dtype=np.float32)
    for ci in range(8):
        pg, cg = ci // CGROUPS, ci % CGROUPS
        out[pg * P:(pg + 1) * P, cg * C:(cg + 1) * C] = res.results[ci]["en"]
    out += const[None, :]
    out -= out.min()
    if _trace:
        kernel._last_results = res
    return out
